# revision 2
# baseline (speedup 1.0000x reference)
"""TRN2 Bass kernel for the ConceptualMambaBlock problem (bf16 redesign).

Math (reference):
    x: [B=4, T=96, N=512, H=128] f32
    expanded = x @ W_exp.T + b_exp            # [B,T,N,2H]
    primary, gating = split(expanded, 2, -1)
    s_t = 0.9*s_{t-1} + 0.1*gating_t          # EMA along T
    out = (primary * sigmoid(s)) @ W_con.T + b_con

Key restructure vs the fp32 baseline: the EMA is linear, so it commutes
with the gating Linear:

    s_t = (0.1*W_g) @ z_t + b_g*(1 - 0.9^t),   z_t = sum_{k<=t} 0.9^{t-k} x_k

  - The scan therefore runs on the *input* x in SBUF at bf16 (fast DVE
    path, batched 4-block ops) instead of on the matmul output in PSUM
    at fp32 (slow 1x path).
  - The bias term splits into +b_g on the sigmoid's per-partition bias
    port plus a rank-1 matmul  b_g (x) (-0.9^t)  accumulated into the
    gating PSUM before the main gating matmul (exact, no ACT fixup).
  - The 0.9^t weights inside the scan use a greedy per-column bf16
    multiplier sequence whose running products track 0.9^j to within
    one bf16 ulp (no compounding of the bf16 rounding of 0.9).

Everything runs in bf16 (inputs, weights, gate, y, output; PSUM stays
fp32 by construction; the scan's internal state is fp32 in HW):
  - PE matmuls at full bf16 rate (~3.5x faster than the fp32r baseline)
  - input + output HBM traffic halved
  - tolerance is 2e-2; measured error of this design is ~1e-3.

Sharding: core c -> batch c//2, node half c%2 (same as baseline).
Layout per core: x^T [H, NLOC, T] -> [H, 64 blocks, 384 cols], col =
(node-in-block, t), t fastest.  Host pre-transposes/casts; host
un-transposes the bf16 output.

PSUM (8 banks of [128, 512] f32) is managed manually in one tile:
  banks 2k, 2k+1   : gating pg(g) for iter parity k; reused for po(g)
                     after the sigmoid has drained pg
  banks 4+2k, 5+2k : primary pp(g) for iter parity k
Per matmul-group iteration g (2 blocks): rank1+mm1g -> pg | sigmoid ->
sg | mm1p -> pp | (next iter) stt -> y | mm2 -> po | Identity+b2 -> ob.
DVE additionally runs the scan for group s+1 (2 chunks of 4 blocks)
interleaved between stt ops.  DMA in/out in 8-block groups (~786 KB).
"""

import numpy as np
import ml_dtypes

import concourse.bacc as bacc
import concourse.mybir as mybir
import concourse.tile as tile
from concourse.bass_utils import run_bass_kernel_spmd

F32 = mybir.dt.float32
BF16 = mybir.dt.bfloat16
AF = mybir.ActivationFunctionType
ALU = mybir.AluOpType

B, T, N, H = 4, 96, 512, 128
NCORES = 8
NLOC = N // 2          # 256 nodes per core
NB = 4                 # nodes per block
TOK = NB * T           # 384 columns per block
NBLK = NLOC // NB      # 64 blocks per core
SG = 8                 # blocks per DMA/scan group
NSG = NBLK // SG       # 8 groups
MG = 2                 # blocks per matmul-group iteration
NMG = NBLK // MG       # 32 iterations
SCH = 4                # blocks per scan chunk (2 chunks per group)

_NC_CACHE = None


def _greedy_mask_pattern():
    """Per-node 96-column multiplier sequence: col 0 is 0.0 (segment
    reset); col j is a bf16 value m_j chosen so prod(m_1..m_j) tracks
    0.9^j to within one bf16 ulp (no error compounding)."""
    ms = [0.0]
    c_act = 1.0
    tgt = 1.0
    for _ in range(1, 96):
        tgt *= 0.9
        m = float(np.asarray(tgt / c_act, dtype=np.float32).astype(ml_dtypes.bfloat16))
        ms.append(m)
        c_act *= m
    return np.array(ms, dtype=np.float64)


def _build():
    nc = bacc.Bacc()

    xt_h = nc.dram_tensor("xt", [H, NBLK, TOK], BF16, kind="ExternalInput")
    wpack_h = nc.dram_tensor("wpack", [H, 3 * H], BF16, kind="ExternalInput")
    rk_h = nc.dram_tensor("rk", [1, H + TOK], BF16, kind="ExternalInput")
    bias_h = nc.dram_tensor("bias", [H, 3], F32, kind="ExternalInput")
    mask_h = nc.dram_tensor("mask", [H, SCH * TOK], BF16, kind="ExternalInput")
    out_h = nc.dram_tensor("out", [H, NBLK, TOK], BF16, kind="ExternalOutput")

    with tile.TileContext(nc) as tc:
        with (
            tc.tile_pool(name="consts", bufs=1) as cp,
            tc.tile_pool(name="io", bufs=1) as io,
            tc.tile_pool(name="mid", bufs=1) as mid,
            tc.tile_pool(name="ps", bufs=1, space="PSUM") as ps,
        ):
            wpack_sb = cp.tile([H, 3 * H], BF16, tag="wpack")
            nc.sync.dma_start(out=wpack_sb[:], in_=wpack_h[:, :])
            rk_sb = cp.tile([1, H + TOK], BF16, tag="rk")
            nc.sync.dma_start(out=rk_sb[:], in_=rk_h[:, :])
            bias_sb = cp.tile([H, 3], F32, tag="bias")
            nc.sync.dma_start(out=bias_sb[:], in_=bias_h[:, :])
            mask_sb = cp.tile([H, SCH * TOK], BF16, tag="mask")
            nc.sync.dma_start(out=mask_sb[:], in_=mask_h[:, :])

            w1pT = wpack_sb[:, 0:H]          # W_exp[:H].T
            w1gT = wpack_sb[:, H : 2 * H]    # (0.1*W_exp[H:]).T
            wcT = wpack_sb[:, 2 * H : 3 * H]  # W_con.T
            bgrow = rk_sb[:, 0:H]            # [1,128] = b_g
            powv = rk_sb[:, H : H + TOK]     # [1,384] = -0.9^(t+1), t-per-node
            bg_ap = bias_sb[:, 0:1]          # sigmoid bias (+b_g)
            b1p_ap = bias_sb[:, 1:2]         # primary bias
            b2_ap = bias_sb[:, 2:3]          # output bias

            psum = ps.tile([H, 8, 512], F32, tag="all")

            state = {}
            xts = [None] * NSG
            zs = [None] * NSG
            obs = [None] * NSG

            def load_group(s):
                xg = io.tile([H, SG, TOK], BF16, tag="x", name=f"x{s}", bufs=3)
                nc.sync.dma_start(out=xg[:], in_=xt_h[:, s * SG : (s + 1) * SG, :])
                xts[s] = xg

            def scan_chunk(s, h):
                if zs[s] is None:
                    zs[s] = mid.tile([H, SG, TOK], BF16, tag="z", name=f"z{s}", bufs=2)
                x2 = xts[s][:, h * SCH : (h + 1) * SCH, :].rearrange("p a b -> p (a b)")
                z2 = zs[s][:, h * SCH : (h + 1) * SCH, :].rearrange("p a b -> p (a b)")
                nc.vector.tensor_tensor_scan(
                    out=z2, data0=mask_sb[:], data1=x2,
                    initial=0.0, op0=ALU.mult, op1=ALU.add,
                )

            def emit_stt(g):
                # y = (pp + b1p) * sg, fused PSUM drain on DVE
                k1 = g % 2
                y_t = mid.tile([H, MG, TOK], BF16, tag="y", name=f"y{g % 4}", bufs=3)
                nc.vector.scalar_tensor_tensor(
                    out=y_t[:],
                    in0=psum[:, 4 + 2 * k1 : 6 + 2 * k1, 0:TOK],
                    scalar=b1p_ap,
                    in1=state[g]["sg"][:],
                    op0=ALU.add, op1=ALU.mult,
                )
                state[g]["y"] = y_t

            def emit_mm2_and_out(g):
                k1 = g % 2
                s1 = g * MG // SG
                q1 = (g * MG % SG) // MG
                y_t = state[g]["y"]
                for j in range(MG):
                    nc.tensor.matmul(
                        psum[:, 2 * k1 + j, 0:TOK], lhsT=wcT, rhs=y_t[:, j, :],
                        start=True, stop=True,
                    )
                nc.scalar.activation(
                    obs[s1][:, q1 * MG : (q1 + 1) * MG, :],
                    psum[:, 2 * k1 : 2 * k1 + 2, 0:TOK],
                    AF.Identity, bias=b2_ap, scale=1.0,
                )
                if q1 == SG // MG - 1:
                    nc.gpsimd.dma_start(
                        out=out_h[:, s1 * SG : (s1 + 1) * SG, :], in_=obs[s1][:]
                    )
                del state[g]

            # prologue: first two input groups + first scan
            load_group(0)
            load_group(1)
            scan_chunk(0, 0)
            scan_chunk(0, 1)

            for g in range(NMG):
                s = g * MG // SG
                q = (g * MG % SG) // MG
                k = g % 2
                if q == 0:
                    if s + 2 < NSG:
                        load_group(s + 2)
                    obs[s] = io.tile([H, SG, TOK], BF16, tag="ob", name=f"ob{s}", bufs=2)

                # PE: rank-1 bias then gating matmul into pg slots
                for j in range(MG):
                    nc.tensor.matmul(
                        psum[:, 2 * k + j, 0:TOK], lhsT=bgrow, rhs=powv,
                        start=True, stop=False,
                    )
                for j in range(MG):
                    nc.tensor.matmul(
                        psum[:, 2 * k + j, 0:TOK], lhsT=w1gT, rhs=zs[s][:, q * MG + j, :],
                        start=False, stop=True,
                    )

                # DVE: previous iteration's gate-mul (deps long ready)
                if g - 1 in state:
                    emit_stt(g - 1)

                # PE: primary matmuls
                for j in range(MG):
                    nc.tensor.matmul(
                        psum[:, 4 + 2 * k + j, 0:TOK], lhsT=w1pT,
                        rhs=xts[s][:, q * MG + j, :],
                        start=True, stop=True,
                    )

                # DVE: scan prefetch for group s+1 (2 chunks, spread out)
                if s + 1 < NSG and q in (1, 2):
                    scan_chunk(s + 1, q - 1)

                # ACT: sigmoid(pg + b_g) -> bf16 gate
                sg_t = mid.tile([H, MG, TOK], BF16, tag="sg", name=f"sg{g % 4}", bufs=3)
                nc.scalar.activation(
                    sg_t[:], psum[:, 2 * k : 2 * k + 2, 0:TOK],
                    AF.Sigmoid, bias=bg_ap, scale=1.0,
                )
                state[g] = {"sg": sg_t}

                # PE: mm2 + ACT outcopy + DMA of g-1 (po reuses pg slots)
                if g - 1 in state and "y" in state[g - 1]:
                    emit_mm2_and_out(g - 1)

            emit_stt(NMG - 1)
            emit_mm2_and_out(NMG - 1)

    nc.finalize()
    return nc


def _get_nc():
    global _NC_CACHE
    if _NC_CACHE is None:
        _NC_CACHE = _build()
    return _NC_CACHE


def _in_maps(x, W_exp, b_exp, W_con, b_con):
    bf16 = ml_dtypes.bfloat16
    wpack = np.concatenate(
        [W_exp[:H, :].T, (0.1 * W_exp[H:, :]).T, W_con.T], axis=1
    ).astype(bf16)
    wpack = np.ascontiguousarray(wpack)

    bg = b_exp[H:]
    tpow = 0.9 ** (np.arange(1, T + 1, dtype=np.float64))
    rk = np.zeros((1, H + TOK), dtype=np.float64)
    rk[0, :H] = bg
    rk[0, H:] = np.tile(-tpow, NB)
    rk = rk.astype(bf16)

    bias = np.stack([bg, b_exp[:H], b_con], axis=1).astype(np.float32)
    bias = np.ascontiguousarray(bias)

    mpat = _greedy_mask_pattern()                    # [96]
    mask = np.tile(mpat, SCH * NB)[None, :].repeat(H, axis=0).astype(bf16)
    mask = np.ascontiguousarray(mask)

    maps = []
    for c in range(NCORES):
        bb, nh = c // 2, c % 2
        xs = x[bb, :, nh * NLOC : (nh + 1) * NLOC, :]  # [T, NLOC, H]
        xT = np.ascontiguousarray(xs.transpose(2, 1, 0)).astype(bf16)
        maps.append(
            {
                "xt": xT.reshape(H, NBLK, TOK),
                "wpack": wpack,
                "rk": rk,
                "bias": bias,
                "mask": mask,
            }
        )
    return maps


def run_spmd(x, W_exp, b_exp, W_con, b_con, **spmd_kwargs):
    """Run the 8-core kernel; returns (full_output, BassKernelResults)."""
    maps = _in_maps(x, W_exp, b_exp, W_con, b_con)
    res = run_bass_kernel_spmd(
        _get_nc(), maps, core_ids=list(range(NCORES)), **spmd_kwargs
    )
    out = np.empty((B, T, N, H), dtype=np.float32)
    for c in range(NCORES):
        bb, nh = c // 2, c % 2
        oT = res.results[c]["out"].astype(np.float32).reshape(H, NLOC, T)
        out[bb, :, nh * NLOC : (nh + 1) * NLOC, :] = oT.transpose(2, 1, 0)
    return out, res


def kernel(spatial_temporal_representation, W_exp, b_exp, W_con, b_con):
    out, _ = run_spmd(
        np.asarray(spatial_temporal_representation, dtype=np.float32),
        np.asarray(W_exp, dtype=np.float32),
        np.asarray(b_exp, dtype=np.float32),
        np.asarray(W_con, dtype=np.float32),
        np.asarray(b_con, dtype=np.float32),
    )
    return out


# revision 6
# speedup vs baseline: 1.0506x; 1.0506x over previous
"""TRN2 Bass kernel for the ConceptualMambaBlock problem (bf16 redesign).

Math (reference):
    x: [B=4, T=96, N=512, H=128] f32
    expanded = x @ W_exp.T + b_exp            # [B,T,N,2H]
    primary, gating = split(expanded, 2, -1)
    s_t = 0.9*s_{t-1} + 0.1*gating_t          # EMA along T
    out = (primary * sigmoid(s)) @ W_con.T + b_con

Key restructure vs the fp32 baseline: the EMA is linear, so it commutes
with the gating Linear:

    s_t = (0.1*W_g) @ z_t + b_g*(1 - 0.9^t),   z_t = sum_{k<=t} 0.9^{t-k} x_k

  - The scan therefore runs on the *input* x in SBUF at bf16 (fast DVE
    path, batched 4-block ops) instead of on the matmul output in PSUM
    at fp32 (slow 1x path).
  - The bias term splits into +b_g on the sigmoid's per-partition bias
    port plus a rank-1 matmul  b_g (x) (-0.9^t)  accumulated into the
    gating PSUM before the main gating matmul (exact, no ACT fixup).
  - The 0.9^t weights inside the scan use a greedy per-column bf16
    multiplier sequence whose running products track 0.9^j to within
    one bf16 ulp (no compounding of the bf16 rounding of 0.9).

Everything runs in bf16 (inputs, weights, gate, y, output; PSUM stays
fp32 by construction; the scan's internal state is fp32 in HW):
  - PE matmuls at full bf16 rate (~3.5x faster than the fp32r baseline)
  - input + output HBM traffic halved
  - tolerance is 2e-2; measured error of this design is ~1e-3.

Sharding: core c -> batch c//2, node half c%2 (same as baseline).
Layout per core: x^T [H, NLOC, T] -> [H, 64 blocks, 384 cols], col =
(node-in-block, t), t fastest.  Host pre-transposes/casts; host
un-transposes the bf16 output.

PSUM (8 banks of [128, 512] f32) is managed manually in one tile:
  banks 2k, 2k+1   : gating pg(g) for iter parity k; reused for po(g)
                     after the sigmoid has drained pg
  banks 4+2k, 5+2k : primary pp(g) for iter parity k
Per matmul-group iteration g (2 blocks): rank1+mm1g -> pg | sigmoid ->
sg | mm1p -> pp | (next iter) stt -> y | mm2 -> po | Identity+b2 -> ob.
DVE additionally runs the scan for group s+1 (2 chunks of 4 blocks)
interleaved between stt ops.  DMA in/out in 8-block groups (~786 KB).
"""

import numpy as np
import ml_dtypes

import concourse.bacc as bacc
import concourse.mybir as mybir
import concourse.tile as tile
from concourse.bass_utils import run_bass_kernel_spmd

F32 = mybir.dt.float32
BF16 = mybir.dt.bfloat16
AF = mybir.ActivationFunctionType
ALU = mybir.AluOpType

B, T, N, H = 4, 96, 512, 128
NCORES = 8
NLOC = N // 2          # 256 nodes per core
NB = 4                 # nodes per block
TOK = NB * T           # 384 columns per block
NBLK = NLOC // NB      # 64 blocks per core
SG = 8                 # blocks per DMA/scan group
NSG = NBLK // SG       # 8 groups
MG = 2                 # blocks per matmul-group iteration
NMG = NBLK // MG       # 32 iterations
SCH = 4                # blocks per scan chunk (2 chunks per group)

_NC_CACHE = None


def _greedy_mask_pattern():
    """Per-node 96-column multiplier sequence: col 0 is 0.0 (segment
    reset); col j is a bf16 value m_j chosen so prod(m_1..m_j) tracks
    0.9^j to within one bf16 ulp (no error compounding)."""
    ms = [0.0]
    c_act = 1.0
    tgt = 1.0
    for _ in range(1, 96):
        tgt *= 0.9
        m = float(np.asarray(tgt / c_act, dtype=np.float32).astype(ml_dtypes.bfloat16))
        ms.append(m)
        c_act *= m
    return np.array(ms, dtype=np.float64)


def _build():
    nc = bacc.Bacc()

    xt_h = nc.dram_tensor("xt", [H, NBLK, TOK], BF16, kind="ExternalInput")
    wpack_h = nc.dram_tensor("wpack", [H, 3 * H], BF16, kind="ExternalInput")
    rk_h = nc.dram_tensor("rk", [1, H + TOK], BF16, kind="ExternalInput")
    bias_h = nc.dram_tensor("bias", [H, 3], F32, kind="ExternalInput")
    mask_h = nc.dram_tensor("mask", [H, SCH * TOK], BF16, kind="ExternalInput")
    out_h = nc.dram_tensor("out", [H, NBLK, TOK], BF16, kind="ExternalOutput")

    with tile.TileContext(nc) as tc:
        with (
            tc.tile_pool(name="consts", bufs=1) as cp,
            tc.tile_pool(name="io", bufs=1) as io,
            tc.tile_pool(name="mid", bufs=1) as mid,
            tc.tile_pool(name="ps", bufs=1, space="PSUM") as ps,
        ):
            wpack_sb = cp.tile([H, 3 * H], BF16, tag="wpack")
            nc.sync.dma_start(out=wpack_sb[:], in_=wpack_h[:, :])
            rk_sb = cp.tile([1, H + TOK], BF16, tag="rk")
            nc.sync.dma_start(out=rk_sb[:], in_=rk_h[:, :])
            bias_sb = cp.tile([H, 3], F32, tag="bias")
            nc.sync.dma_start(out=bias_sb[:], in_=bias_h[:, :])
            mask_sb = cp.tile([H, SCH * TOK], BF16, tag="mask")
            nc.sync.dma_start(out=mask_sb[:], in_=mask_h[:, :])

            w1pT = wpack_sb[:, 0:H]          # W_exp[:H].T
            w1gT = wpack_sb[:, H : 2 * H]    # (0.1*W_exp[H:]).T
            wcT = wpack_sb[:, 2 * H : 3 * H]  # W_con.T
            bgrow = rk_sb[:, 0:H]            # [1,128] = b_g
            powv = rk_sb[:, H : H + TOK]     # [1,384] = -0.9^(t+1), t-per-node
            bg_ap = bias_sb[:, 0:1]          # sigmoid bias (+b_g)
            b1p_ap = bias_sb[:, 1:2]         # primary bias
            b2_ap = bias_sb[:, 2:3]          # output bias

            psum = ps.tile([H, 8, 512], F32, tag="all")

            # PE warm-up: ~12 back-to-back matmuls (~4us cold) during the
            # initial input-DMA wait push the PE HAM to K=8/8 (2.4 GHz);
            # everything after runs at the warm rate. Uses const tiles as
            # operands; scratch dst in bank 6 (first real use is >6us in).
            for _ in range(12):
                nc.tensor.matmul(
                    psum[:, 6, 0:TOK], lhsT=wpack_sb[:, 0:H],
                    rhs=mask_sb[:, 0:TOK], start=True, stop=True,
                )

            state = {}
            xts = [None] * NSG
            zs = [None] * NSG
            obs = [None] * NSG

            def load_group(s):
                xg = io.tile([H, SG, TOK], BF16, tag="x", name=f"x{s}", bufs=3)
                nc.sync.dma_start(out=xg[:], in_=xt_h[:, s * SG : (s + 1) * SG, :])
                xts[s] = xg

            def scan_chunk(s, h):
                if zs[s] is None:
                    zs[s] = mid.tile([H, SG, TOK], BF16, tag="z", name=f"z{s}", bufs=2)
                x2 = xts[s][:, h * SCH : (h + 1) * SCH, :].rearrange("p a b -> p (a b)")
                z2 = zs[s][:, h * SCH : (h + 1) * SCH, :].rearrange("p a b -> p (a b)")
                nc.vector.tensor_tensor_scan(
                    out=z2, data0=mask_sb[:], data1=x2,
                    initial=0.0, op0=ALU.mult, op1=ALU.add,
                )

            def emit_stt(g):
                # y = (pp + b1p) * sg, fused PSUM drain on DVE
                k1 = g % 2
                y_t = mid.tile([H, MG, TOK], BF16, tag="y", name=f"y{g % 4}", bufs=3)
                nc.vector.scalar_tensor_tensor(
                    out=y_t[:],
                    in0=psum[:, 4 + 2 * k1 : 6 + 2 * k1, 0:TOK],
                    scalar=b1p_ap,
                    in1=state[g]["sg"][:],
                    op0=ALU.add, op1=ALU.mult,
                )
                state[g]["y"] = y_t

            def emit_mm2_and_out(g):
                k1 = g % 2
                s1 = g * MG // SG
                q1 = (g * MG % SG) // MG
                y_t = state[g]["y"]
                for j in range(MG):
                    nc.tensor.matmul(
                        psum[:, 2 * k1 + j, 0:TOK], lhsT=wcT, rhs=y_t[:, j, :],
                        start=True, stop=True,
                    )
                nc.scalar.activation(
                    obs[s1][:, q1 * MG : (q1 + 1) * MG, :],
                    psum[:, 2 * k1 : 2 * k1 + 2, 0:TOK],
                    AF.Identity, bias=b2_ap, scale=1.0,
                )
                if q1 == SG // MG - 1:
                    # scalar-issued HWDGE ring: separate from the sync input
                    # ring, and keeps the gpsimd queue free for scan chunks
                    nc.scalar.dma_start(
                        out=out_h[:, s1 * SG : (s1 + 1) * SG, :], in_=obs[s1][:]
                    )
                del state[g]

            # prologue: first two input groups + first scan
            load_group(0)
            load_group(1)
            scan_chunk(0, 0)
            scan_chunk(0, 1)

            for g in range(NMG):
                s = g * MG // SG
                q = (g * MG % SG) // MG
                k = g % 2
                if q == 0:
                    if s + 2 < NSG:
                        load_group(s + 2)
                    obs[s] = io.tile([H, SG, TOK], BF16, tag="ob", name=f"ob{s}", bufs=2)

                # PE: rank-1 bias then gating matmul into pg slots
                for j in range(MG):
                    nc.tensor.matmul(
                        psum[:, 2 * k + j, 0:TOK], lhsT=bgrow, rhs=powv,
                        start=True, stop=False,
                    )
                for j in range(MG):
                    nc.tensor.matmul(
                        psum[:, 2 * k + j, 0:TOK], lhsT=w1gT, rhs=zs[s][:, q * MG + j, :],
                        start=False, stop=True,
                    )

                # DVE: previous iteration's gate-mul (deps long ready)
                if g - 1 in state:
                    emit_stt(g - 1)

                # PE: primary matmuls
                for j in range(MG):
                    nc.tensor.matmul(
                        psum[:, 4 + 2 * k + j, 0:TOK], lhsT=w1pT,
                        rhs=xts[s][:, q * MG + j, :],
                        start=True, stop=True,
                    )

                # DVE: scan prefetch for group s+1 (2 chunks, spread out)
                if s + 1 < NSG and q in (1, 2):
                    scan_chunk(s + 1, q - 1)

                # ACT: sigmoid(pg + b_g) -> bf16 gate
                sg_t = mid.tile([H, MG, TOK], BF16, tag="sg", name=f"sg{g % 4}", bufs=3)
                nc.scalar.activation(
                    sg_t[:], psum[:, 2 * k : 2 * k + 2, 0:TOK],
                    AF.Sigmoid, bias=bg_ap, scale=1.0,
                )
                state[g] = {"sg": sg_t}

                # PE: mm2 + ACT outcopy + DMA of g-1 (po reuses pg slots)
                if g - 1 in state and "y" in state[g - 1]:
                    emit_mm2_and_out(g - 1)

            emit_stt(NMG - 1)
            emit_mm2_and_out(NMG - 1)

    nc.finalize()
    return nc


def _get_nc():
    global _NC_CACHE
    if _NC_CACHE is None:
        _NC_CACHE = _build()
    return _NC_CACHE


def _in_maps(x, W_exp, b_exp, W_con, b_con):
    bf16 = ml_dtypes.bfloat16
    wpack = np.concatenate(
        [W_exp[:H, :].T, (0.1 * W_exp[H:, :]).T, W_con.T], axis=1
    ).astype(bf16)
    wpack = np.ascontiguousarray(wpack)

    bg = b_exp[H:]
    tpow = 0.9 ** (np.arange(1, T + 1, dtype=np.float64))
    rk = np.zeros((1, H + TOK), dtype=np.float64)
    rk[0, :H] = bg
    rk[0, H:] = np.tile(-tpow, NB)
    rk = rk.astype(bf16)

    bias = np.stack([bg, b_exp[:H], b_con], axis=1).astype(np.float32)
    bias = np.ascontiguousarray(bias)

    mpat = _greedy_mask_pattern()                    # [96]
    mask = np.tile(mpat, SCH * NB)[None, :].repeat(H, axis=0).astype(bf16)
    mask = np.ascontiguousarray(mask)

    maps = []
    for c in range(NCORES):
        bb, nh = c // 2, c % 2
        xs = x[bb, :, nh * NLOC : (nh + 1) * NLOC, :]  # [T, NLOC, H]
        xT = np.ascontiguousarray(xs.transpose(2, 1, 0)).astype(bf16)
        maps.append(
            {
                "xt": xT.reshape(H, NBLK, TOK),
                "wpack": wpack,
                "rk": rk,
                "bias": bias,
                "mask": mask,
            }
        )
    return maps


def run_spmd(x, W_exp, b_exp, W_con, b_con, **spmd_kwargs):
    """Run the 8-core kernel; returns (full_output, BassKernelResults)."""
    maps = _in_maps(x, W_exp, b_exp, W_con, b_con)
    res = run_bass_kernel_spmd(
        _get_nc(), maps, core_ids=list(range(NCORES)), **spmd_kwargs
    )
    out = np.empty((B, T, N, H), dtype=np.float32)
    for c in range(NCORES):
        bb, nh = c // 2, c % 2
        oT = res.results[c]["out"].astype(np.float32).reshape(H, NLOC, T)
        out[bb, :, nh * NLOC : (nh + 1) * NLOC, :] = oT.transpose(2, 1, 0)
    return out, res


def kernel(spatial_temporal_representation, W_exp, b_exp, W_con, b_con):
    out, _ = run_spmd(
        np.asarray(spatial_temporal_representation, dtype=np.float32),
        np.asarray(W_exp, dtype=np.float32),
        np.asarray(b_exp, dtype=np.float32),
        np.asarray(W_con, dtype=np.float32),
        np.asarray(b_con, dtype=np.float32),
    )
    return out
